# revision 17
# baseline (speedup 1.0000x reference)
"""CBOW negative-sampling loss kernel for 8 Trainium2 NeuronCores.

Math (faithful to the reference, including its [B]+[B,1] broadcast bug):
    c_b   = mean_w ctx_w[context[b, w]]               # [D]
    pos_b = log_sigmoid(emb_w[target[b]] . c_b)
    neg_b = sum_k log_sigmoid(emb_w[noise[b, k]] . c_b)
    out   = -(mean_b pos_b + mean_b neg_b) = -(sum_b (pos_b + neg_b)) / B

Strategy: shard B across the 8 cores (2048 samples each). Per core the host
packs one int32 index matrix; each 256-sample group issues two indirect
(gather) DMAs that also downcast rows to bf16 in flight:
  - ctx rows land one-row-per-partition, sample-major, so the context mean
    is 10 accumulating TensorE matmuls against a static 0/1 pooling matrix
    (PSUM holds c in [sample, D] layout).
  - target+noise rows land 11 segments per sample along partition p's free
    dim; one DVE multiply against broadcast c + one strided reduce gives all
    11 dots per sample.
Sigmoid(0.1*x) + Ln on the scalar engine (Ln's accum_out) yield each
sample's summed log-sigmoid. Host sums the [128, n_groups] per-core partials
and scales by -1/B.
"""

import numpy as np

V, D = 100000, 128
B, W, K = 16384, 10, 10
NCORES = 8
P = 128
B_LOCAL = B // NCORES  # 2048
NBLK = B_LOCAL // P  # 16 blocks of 128 samples
GB = 4  # blocks per gather group
NGRP = NBLK // GB  # 8 groups
SEG = W + 1 + K  # 21 rows gathered per sample
CTX_COLS = GB * W  # 20 ctx gather slots per group
EMB_COLS = GB * (K + 1)  # 22 emb gather slots per group
GSEG = CTX_COLS + EMB_COLS  # 42 index columns per group

_LAST_RESULTS = None  # test harness introspection (exec_time_ns etc.)


def _build_bass(ngrp, gb, vocab):
    import concourse.bass as bass
    import concourse.tile as tile
    from concourse import bacc, mybir

    w, k = W, K
    kp1 = k + 1
    ctx_cols = gb * w
    emb_cols = gb * kp1
    gseg = ctx_cols + emb_cols
    nc = bacc.Bacc(None, target_bir_lowering=False)
    idx_d = nc.declare_dram_parameter(
        "idx", [P, ngrp * gseg], mybir.dt.int32, isOutput=False
    )
    pool_d = nc.declare_dram_parameter(
        "pool", [P, w * P], mybir.dt.bfloat16, isOutput=False
    )
    ctx_w_d = nc.declare_dram_parameter(
        "ctx_w", [vocab, D], mybir.dt.bfloat16, isOutput=False
    )
    emb_w_d = nc.declare_dram_parameter(
        "emb_w", [vocab, D], mybir.dt.bfloat16, isOutput=False
    )
    out_d = nc.declare_dram_parameter("out", [P, 1], mybir.dt.float32, isOutput=True)

    with tile.TileContext(nc) as tc:
        with (
            tc.tile_pool(name="const", bufs=1) as cpool,
            tc.tile_pool(name="gather", bufs=4) as gpool,
            tc.tile_pool(name="work", bufs=3) as wpool,
            tc.tile_pool(name="psum", bufs=4, space="PSUM") as ppool,
        ):
            idx_sb = cpool.tile([P, ngrp * gseg], mybir.dt.int32)
            nc.sync.dma_start(out=idx_sb[:], in_=idx_d[:])
            pool_sb = cpool.tile([P, w * P], mybir.dt.bfloat16)
            nc.sync.dma_start(out=pool_sb[:], in_=pool_d[:])
            all_dots = cpool.tile([P, ngrp * gb * kp1], mybir.dt.bfloat16)
            acc = cpool.tile([P, 1], mybir.dt.float32)

            for g in range(ngrp):
                c0 = g * gseg
                # ctx rows, one per partition, sample-major: slot t holds rows
                # t*128..t*128+127 of this group's 2560 ctx rows.
                Tctx = gpool.tile([P, ctx_cols * D], mybir.dt.bfloat16, tag="Tctx")
                nc.gpsimd.indirect_dma_start(
                    out=Tctx[:],
                    out_offset=None,
                    in_=ctx_w_d[:],
                    in_offset=bass.IndirectOffsetOnAxis(
                        ap=idx_sb[:, c0 : c0 + ctx_cols], axis=0
                    ),
                )
                # target+noise rows: per block b, 11 segments per sample along
                # partition p's free dim ([b][tgt, noise*10][D]).
                Temb = gpool.tile([P, emb_cols * D], mybir.dt.bfloat16, tag="Temb")
                nc.gpsimd.indirect_dma_start(
                    out=Temb[:],
                    out_offset=None,
                    in_=emb_w_d[:],
                    in_offset=bass.IndirectOffsetOnAxis(
                        ap=idx_sb[:, c0 + ctx_cols : c0 + gseg], axis=0
                    ),
                )

                c_sb = wpool.tile([P, gb * D], mybir.dt.bfloat16, tag="c")
                dots = all_dots[:, g * gb * kp1 : (g + 1) * gb * kp1]
                # c_raw[s, d(+block)] = sum_r pool[r, s] * ctx_rows[r, d] on
                # TensorE; slot u holds both blocks' tiles side by side so one
                # 256-wide matmul per u covers the whole group.
                c_ps = ppool.tile([P, gb * D], mybir.dt.float32, tag="cps")
                for u in range(w):
                    nc.tensor.matmul(
                        c_ps[:],
                        lhsT=pool_sb[:, u * P : (u + 1) * P],
                        rhs=Tctx[:, u * gb * D : (u + 1) * gb * D],
                        start=(u == 0),
                        stop=(u == w - 1),
                    )
                nc.scalar.activation(
                    out=c_sb[:],
                    in_=c_ps[:],
                    func=mybir.ActivationFunctionType.Copy,
                )
                # all 11*gb dots in one multiply + one strided reduce
                prod = wpool.tile([P, gb * kp1 * D], mybir.dt.bfloat16, tag="prod")
                nc.vector.tensor_tensor(
                    out=prod[:],
                    in0=Temb[:],
                    in1=c_sb[:]
                    .rearrange("p (b d) -> p b d", b=gb)
                    .unsqueeze(2)
                    .broadcast_to([P, gb, kp1, D]),
                    op=mybir.AluOpType.mult,
                )
                with nc.allow_low_precision("dots are ~1e-4; bf16 out keeps DVE 2x"):
                    nc.vector.tensor_reduce(
                        out=dots[:],
                        in_=prod[:].rearrange("p (s d) -> p s d", s=gb * kp1),
                        axis=mybir.AxisListType.X,
                        op=mybir.AluOpType.add,
                    )

            # One tail pass: log-sigmoid of all true dots (0.1 rescales the
            # ctx sum to a mean); Ln's accum_out emits per-partition sums.
            sig = cpool.tile([P, ngrp * gb * kp1], mybir.dt.float32)
            nc.scalar.activation(
                out=sig[:],
                in_=all_dots[:],
                func=mybir.ActivationFunctionType.Sigmoid,
                scale=1.0 / w,
            )
            ls = cpool.tile([P, ngrp * gb * kp1], mybir.dt.float32)
            nc.scalar.activation(
                out=ls[:],
                in_=sig[:],
                func=mybir.ActivationFunctionType.Ln,
                accum_out=acc[:, 0:1],
            )

            nc.sync.dma_start(out=out_d[:], in_=acc[:])
    nc.compile()
    return nc


def _make_pool_matrix():
    """[P, W*P] bf16: pool[r, u*P + s] = 1 iff row u*128+r belongs to sample s."""
    import ml_dtypes

    pool = np.zeros((P, W * P), dtype=np.float32)
    for u in range(W):
        for r in range(P):
            s = (u * P + r) // W  # sample-in-block, < 128
            pool[r, u * P + s] = 1.0
    return pool.astype(ml_dtypes.bfloat16)


def _pack_indices(context, target, noise, ncores, nblk, gb):
    """Per-core [P, ngrp*GSEG] int32 index matrices in gather layout."""
    ngrp = nblk // gb
    spg = gb * P  # samples per group
    ctx_cols = gb * W
    kp1 = K + 1
    ctx_r = np.ascontiguousarray(context, dtype=np.int32).reshape(ncores, ngrp, spg, W)
    tgt_r = np.ascontiguousarray(target, dtype=np.int32).reshape(ncores, ngrp, gb, P)
    noi_r = np.ascontiguousarray(noise, dtype=np.int32).reshape(ncores, ngrp, gb, P, K)
    idxs = []
    for n in range(ncores):
        cols = []
        for g in range(ngrp):
            # ctx: slot u*gb+b holds group-rows b*1280+u*128 .. +127 (so both
            # blocks' tiles for pooling-slot u sit side by side)
            flat = ctx_r[n, g].reshape(spg * W)  # ordered (sample, word)
            ctx_part = (
                flat.reshape(gb, W, P).transpose(1, 0, 2).reshape(ctx_cols, P).T
            )
            # emb: per block, [tgt, noise*10] per sample
            emb_part = np.concatenate(
                [
                    np.concatenate(
                        [tgt_r[n, g, b][:, None], noi_r[n, g, b]], axis=1
                    )  # [P, 11]
                    for b in range(gb)
                ],
                axis=1,
            )  # [P, gb*11]
            cols.append(np.concatenate([ctx_part, emb_part], axis=1))
        idxs.append(np.ascontiguousarray(np.concatenate(cols, axis=1)))
    return idxs


def kernel(context, target, noise, emb_w, ctx_w):
    global _LAST_RESULTS
    import os
    import sys

    for p in ("/root/.axon_site/_ro/trn_rl_repo", "/opt/trn_rl_repo"):
        if p not in sys.path:
            sys.path.insert(0, p)
    from concourse.bass_utils import run_bass_kernel_spmd

    import ml_dtypes

    context = np.asarray(context)
    target = np.asarray(target)
    noise = np.asarray(noise)
    bf16 = ml_dtypes.bfloat16
    emb_w = np.ascontiguousarray(np.asarray(emb_w, dtype=np.float32).astype(bf16))
    ctx_w = np.ascontiguousarray(np.asarray(ctx_w, dtype=np.float32).astype(bf16))

    nc = _build_bass(NGRP, GB, V)
    idxs = _pack_indices(context, target, noise, NCORES, NBLK, GB)
    pool = _make_pool_matrix()
    in_maps = [
        {"idx": idxs[n], "pool": pool, "ctx_w": ctx_w, "emb_w": emb_w}
        for n in range(NCORES)
    ]
    tmpdir = os.environ.get("KERNEL_TMPDIR") or None
    res = run_bass_kernel_spmd(nc, in_maps, list(range(NCORES)), tmpdir=tmpdir)
    _LAST_RESULTS = res
    total = sum(
        float(np.sum(np.asarray(r["out"], dtype=np.float64))) for r in res.results
    )
    return np.float32(-total / B)
